# revision 36
# baseline (speedup 1.0000x reference)
"""Trainium2 Bass kernel for nn_BasalGanglia (8-core SPMD).

Math notes (faithful rewrite of the reference):
  - wlat_gpe = EPS*(ones-eye)            so x @ wlat_gpe = EPS*(rowsum(x) - x)
  - wlat_stn = EPS*(ones-eye) + ones     so v @ wlat_stn = (1+EPS)*rowsum(v) - EPS*v
    -> the two (D,D) lateral matmuls collapse to row-reductions + elementwise.
  - The 20-step feed-forward striatal loop V' = J + V*(1-J-K) has closed form
    V_20 = J*(1-G^20)/(J+K) with G = 1-J-K (V_0 = 0).
  - V_GPi is a linear recurrence V' = 0.9 V + 0.1(-DP + 2*lam*P_t); summed in
    closed form: Ithal = (1-0.9^50)*DP - 0.2*lam*sum_t 0.9^(50-t) P_t.

Sharding: phase 1 (stimulus @ W gates, 8.6 GFLOP) is sharded over the output
columns D (256 per core, full batch); the recurrent phase is sharded over the
batch (16 samples per core, full D) with layout [partition = d_hi*16 + b_local,
free = d_lo], d = d_hi*256 + d_lo.  A single on-device AllToAll reshards the
striatal outputs between the phases; its output rows map 1:1 onto partitions.
"""

import sys

if "/opt/trn_rl_repo" not in sys.path:
    sys.path.insert(0, "/opt/trn_rl_repo")

import numpy as np

CORES = 8
B = 128          # batch
BC = B // CORES  # samples per core in phase 2
CTX = 4096
D = 2048
DS = D // CORES  # d-columns per core in phase 1
KT = CTX // 128  # contraction k-tiles
EPS = 0.005
FF_STEPS = 20
NITER = 50

_CACHE = {}


def _build_program(niter=NITER, a2a=True, ablate=frozenset()):
    import concourse.bass as bass
    import concourse.mybir as mybir
    import concourse.tile as tile
    from concourse import bacc

    dt = mybir.dt
    f32 = dt.float32
    f32r = dt.float32r
    OP = mybir.AluOpType
    AF = mybir.ActivationFunctionType

    nc = bacc.Bacc(
        "TRN2",
        target_bir_lowering=False,
        debug=False,
        enable_asserts=False,
        num_devices=CORES,
    )

    stimT_d = nc.dram_tensor("stimT", (CTX, B), f32, kind="ExternalInput").ap()
    wcat_d = nc.dram_tensor("wcat", (CTX, 4 * DS), f32, kind="ExternalInput").ap()
    consts_d = nc.dram_tensor("consts", (128, 9 * 128), f32, kind="ExternalInput").ap()
    wgpi_d = nc.dram_tensor("wgpi", (128, 4 * DS), f32, kind="ExternalInput").ap()
    dvf_d = nc.dram_tensor("dvf", (128, 1), f32, kind="ExternalInput").ap()
    out_d = nc.dram_tensor("out_ithal", (BC, 2), f32, kind="ExternalOutput").ap()

    # const column offsets within consts_d: block-diag group matrices (fp32,
    # used by tiny N<=2 matmuls) then scaled identities (f32r, full streams)
    (C_A, C_AEPS, C_ASV, C_AEN23, C_I2, C_INEPS, C_I23, C_IX23E, C_INVC) = range(9)

    def mm(out, lhsT, rhs, start, stop):
        nc.tensor.matmul(out, lhsT, rhs, start=start, stop=stop)

    with tile.TileContext(nc) as tc:
        with (
            tc.tile_pool(name="cst", bufs=1) as cst,
            tc.tile_pool(name="state", bufs=1) as st,
            tc.tile_pool(name="ldw", bufs=2) as ldw,
            tc.tile_pool(name="wk", bufs=2) as wk,
            tc.tile_pool(name="ff", bufs=1) as ffp,
            tc.tile_pool(name="dram", bufs=1, space="DRAM") as dram,
            tc.tile_pool(name="ps1", bufs=1, space="PSUM") as ps1,
            tc.tile_pool(name="psl", bufs=1, space="PSUM") as psl,
            tc.tile_pool(name="pst", bufs=1, space="PSUM") as pst,
        ):
            # ---------------- constants / small inputs ----------------
            constsA = cst.tile([128, 4 * 128], f32)
            nc.sync.dma_start(constsA[:], consts_d[:, 0 : 4 * 128])
            constsI = cst.tile([128, 5 * 128], f32r)
            nc.sync.dma_start(constsI[:], consts_d[:, 4 * 128 :].bitcast(f32r))

            def cc(i):
                if i < 4:
                    return constsA[:, i * 128 : (i + 1) * 128]
                return constsI[:, (i - 4) * 128 : (i - 3) * 128]

            wgpi = cst.tile([128, 4 * DS], f32)
            nc.sync.dma_start(wgpi[:], wgpi_d[:])
            w1g0 = wgpi[:, 0 * DS : 1 * DS]
            w1g1 = wgpi[:, 1 * DS : 2 * DS]
            w2g0 = wgpi[:, 2 * DS : 3 * DS]
            w2g1 = wgpi[:, 3 * DS : 4 * DS]

            lamdv = st.tile([128, 1], f32)
            nc.sync.dma_start(lamdv[:], dvf_d[:])
            lam = st.tile([128, 1], f32)
            nc.scalar.activation(lam[:], lamdv[:], AF.Sigmoid)

            # ---------------- phase 1: J/K gate matmuls ----------------
            stim_all = cst.tile([128, KT * B], f32r)
            nc.sync.dma_start(
                stim_all[:].rearrange("p (c b) -> p c b", c=KT),
                stimT_d.rearrange("(c p) b -> p c b", c=KT, p=128).bitcast(f32r),
            )

            psJ12 = ps1.tile([128, 512], f32)  # [J1 | K1]
            psJ34 = ps1.tile([128, 512], f32)  # [J2 | K2]

            NWCHUNK = 8
            KT_PER = KT // NWCHUNK  # 4 k-tiles per DMA chunk
            for ch in range(NWCHUNK):
                wt = ldw.tile([128, KT_PER * 4 * DS], f32r, tag="wld")
                nc.sync.dma_start(
                    wt[:].rearrange("p (c w) -> p c w", c=KT_PER),
                    wcat_d[ch * KT_PER * 128 : (ch + 1) * KT_PER * 128, :].rearrange(
                        "(c p) w -> p c w", c=KT_PER, p=128
                    ).bitcast(f32r),
                )
                for i in range(KT_PER):
                    kt = ch * KT_PER + i
                    lhsT = stim_all[:, kt * B : (kt + 1) * B]
                    rhs = wt[:, i * 4 * DS : (i + 1) * 4 * DS]
                    mm(psJ12[:], lhsT, rhs[:, 0:512], start=(kt == 0), stop=(kt == KT - 1))
                    mm(psJ34[:], lhsT, rhs[:, 512:1024], start=(kt == 0), stop=(kt == KT - 1))

            # ---------------- feed-forward loop (closed form) ----------------
            jk12 = ffp.tile([128, 512], f32)
            nc.vector.tensor_copy(jk12[:], psJ12[:])
            jk34 = ffp.tile([128, 512], f32)
            nc.vector.tensor_copy(jk34[:], psJ34[:])
            v_sb = []  # V_D1, V_D2 slices [128(b), 256(d_lo)]
            for J, K in ((jk12[:, 0:DS], jk12[:, DS:512]), (jk34[:, 0:DS], jk34[:, DS:512])):
                jk = ffp.tile([128, DS], f32, tag=f"jk{len(v_sb)}")
                nc.vector.tensor_tensor(jk[:], J, K, OP.add)
                g = ffp.tile([128, DS], f32, tag=f"g{len(v_sb)}")
                nc.vector.tensor_scalar(g[:], jk[:], -1.0, 1.0, OP.mult, OP.add)
                rjk = ffp.tile([128, DS], f32, tag=f"rjk{len(v_sb)}")
                nc.vector.reciprocal(rjk[:], jk[:])
                q = ffp.tile([128, DS], f32, tag=f"q{len(v_sb)}")
                nc.vector.tensor_tensor(q[:], J, rjk[:], OP.mult)
                g2 = ffp.tile([128, DS], f32, tag=f"g2_{len(v_sb)}")
                nc.vector.tensor_tensor(g2[:], g[:], g[:], OP.mult)
                g4 = ffp.tile([128, DS], f32, tag=f"g4_{len(v_sb)}")
                nc.vector.tensor_tensor(g4[:], g2[:], g2[:], OP.mult)
                g8 = ffp.tile([128, DS], f32, tag=f"g8_{len(v_sb)}")
                nc.vector.tensor_tensor(g8[:], g4[:], g4[:], OP.mult)
                g16 = ffp.tile([128, DS], f32, tag=f"g16_{len(v_sb)}")
                nc.vector.tensor_tensor(g16[:], g8[:], g8[:], OP.mult)
                g20 = ffp.tile([128, DS], f32, tag=f"g20_{len(v_sb)}")
                nc.vector.tensor_tensor(g20[:], g16[:], g4[:], OP.mult)
                t = ffp.tile([128, DS], f32, tag=f"t{len(v_sb)}")
                nc.vector.tensor_scalar(t[:], g20[:], -1.0, 1.0, OP.mult, OP.add)
                v = ffp.tile([128, DS], f32, tag=f"v{len(v_sb)}")
                nc.vector.tensor_tensor(v[:], q[:], t[:], OP.mult)
                v_sb.append(v)

            # ---------------- reshard: AllToAll over batch ----------------
            a2a_in = dram.tile([128, 2 * DS], f32)
            a2a_out = dram.tile([128, 2 * DS], f32)
            nc.sync.dma_start(a2a_in[:, 0:DS], v_sb[0][:])
            nc.sync.dma_start(a2a_in[:, DS : 2 * DS], v_sb[1][:])
            if a2a:
                nc.gpsimd.collective_compute(
                    "AllToAll",
                    mybir.AluOpType.bypass,
                    replica_groups=[list(range(CORES))],
                    ins=[a2a_in.opt()],
                    outs=[a2a_out.opt()],
                )
            else:  # timing-model variant: same volume, no collective
                nc.sync.dma_start(a2a_out[:], a2a_in[:])

            # phase-2 layout: partition p = d_hi*16 + b_local, free = d_lo.
            # a2a_out rows are exactly that layout already: row 16*s + i =
            # (sample i of mine, d-slice s from core s).
            v12 = st.tile([128, 2 * DS], f32r)
            nc.sync.dma_start(v12[:], a2a_out[:].bitcast(f32r))
            v1t = v12[:, 0:DS].bitcast(f32)
            v2raw = v12[:, DS : 2 * DS]  # f32r view for PE streaming

            c1 = st.tile([128, DS], f32)  # -V_D2
            nc.vector.tensor_scalar(c1[:], v2raw.bitcast(f32), -1.0, None, OP.mult)

            # ---------------- DP (direct pathway), deferred to epilogue ----
            r56 = st.tile([128, 2], f32)
            dp_s0 = wk.tile([128, DS], f32, tag="dps0")
            nc.vector.scalar_tensor_tensor(
                dp_s0[:], v1t, 1.0, w1g0, OP.mult, OP.mult, accum_out=r56[:, 0:1]
            )
            dp_s1 = wk.tile([128, DS], f32, tag="dps1")
            nc.vector.scalar_tensor_tensor(
                dp_s1[:], v1t, 1.0, w1g1, OP.mult, OP.mult, accum_out=r56[:, 1:2]
            )
            psDP = pst.tile([128, 2], f32, tag="psdp")
            mm(psDP[:], cc(C_A), r56[:], start=True, stop=True)
            dpsb = st.tile([128, 2], f32)
            geo_c = float(1.0 - 0.9**niter)
            nc.vector.tensor_scalar(dpsb[:], psDP[:], geo_c, None, OP.mult)

            # ---------------- STN/GPe recurrent loop ----------------
            Xa = st.tile([128, DS], f32r)
            Xb = st.tile([128, DS], f32r)
            Y = st.tile([128, DS], f32r)
            V = st.tile([128, DS], f32r)
            r1 = st.tile([128, 1], f32)
            r2 = st.tile([128, 1], f32)
            r34 = st.tile([128, 2], f32)
            Aacc = st.tile([128, 2], f32)
            nc.vector.memset(Xa[:].bitcast(f32), 0.0)
            nc.vector.memset(Y[:].bitcast(f32), 0.0)
            nc.vector.memset(V[:].bitcast(f32), 0.0)
            nc.vector.memset(r1[:], 0.0)
            nc.vector.memset(r2[:], 0.0)
            nc.vector.memset(Aacc[:], 0.0)

            for t in range(niter):
                Xold = Xa if t % 2 == 0 else Xb
                Xnew = Xb if t % 2 == 0 else Xa
                # group-sum broadcasts of rowsums (block-diag A stationaries);
                # r1/r2 still hold rowsum(X_old)/rowsum(V_old) here.
                if "tiny" not in ablate:
                    sg = pst.tile([128, 1], f32, tag="sg")  # EPS * Sg_old
                    mm(sg[:], cc(C_AEPS), r1[:], start=True, stop=True)
                    svc = pst.tile([128, 1], f32, tag="svc")
                    mm(svc[:], cc(C_ASV), r2[:], start=True, stop=False)
                    mm(svc[:], cc(C_AEN23), r1[:], start=False, stop=True)
                    sg_ap, svc_ap = sg[:], svc[:]
                    bias_v = wk.tile([128, 1], f32, tag="biasv")
                    nc.vector.tensor_scalar(
                        bias_v[:], svc[:], lam[:], None, OP.mult
                    )
                else:
                    sg_ap, svc_ap = 0.0, 0.0
                    bias_v = None

                # Y' = (2/3)Y + (2EPS/3)X_old - ((4+EPS)/3)V + (2/3)V_D2 + svc
                if "pe" not in ablate:
                    psY = psl.tile([128, DS], f32, tag="psY")
                    mm(psY[:], cc(C_I23), Y[:], start=True, stop=False)
                    mm(psY[:], cc(C_I23), v2raw, start=False, stop=False)
                    mm(psY[:], cc(C_IX23E), Xold[:], start=False, stop=False)
                    mm(psY[:], cc(C_INVC), V[:], start=False, stop=True)

                # X' = 2V - EPS*X_old + EPS*Sg_old + C1   (C1 = -V_D2)
                if "pe" not in ablate:
                    psX = psl.tile([128, DS], f32, tag="psX")
                    mm(psX[:], cc(C_I2), V[:], start=True, stop=False)
                    mm(psX[:], cc(C_INEPS), Xold[:], start=False, stop=True)

                # V' = tanh(lam*(psY + svc)) straight from PSUM; rowsum via
                # accum.  Runs in parallel with both evicts.
                if "act" not in ablate:
                    nc.scalar.activation(
                        V[:], psY[:], AF.Tanh, scale=lam[:],
                        bias=bias_v[:] if bias_v is not None else 0.0,
                        accum_out=r2[:],
                    )
                if "evx" not in ablate:
                    nc.vector.scalar_tensor_tensor(
                        Xnew[:], c1[:], sg_ap, psX[:], OP.add, OP.add,
                        accum_out=r1[:],
                    )
                if "evy" not in ablate:
                    nc.vector.tensor_scalar(
                        Y[:], psY[:], svc_ap, None, OP.add
                    )

                # P_g = rowsum(V' * W2_g); indirect pathway accumulator
                if "p" not in ablate:
                    p_s0 = wk.tile([128, DS], f32, tag="ps0")
                    nc.vector.scalar_tensor_tensor(
                        p_s0[:], V[:], 1.0, w2g0, OP.mult, OP.mult,
                        accum_out=r34[:, 0:1]
                    )
                    p_s1 = wk.tile([128, DS], f32, tag="ps1")
                    nc.vector.scalar_tensor_tensor(
                        p_s1[:], V[:], 1.0, w2g1, OP.mult, OP.mult,
                        accum_out=r34[:, 1:2]
                    )
                    psP = pst.tile([128, 2], f32, tag="psP")
                    mm(psP[:], cc(C_A), r34[:], start=True, stop=True)
                    nc.vector.scalar_tensor_tensor(
                        Aacc[:], Aacc[:], 0.9, psP[:], OP.mult, OP.add
                    )

            # ---------------- epilogue ----------------
            lamneg = st.tile([128, 1], f32)
            nc.vector.tensor_scalar(lamneg[:], lam[:], -0.2, None, OP.mult)
            ith = st.tile([128, 2], f32)
            nc.vector.scalar_tensor_tensor(
                ith[:], Aacc[:], lamneg[:], dpsb[:], OP.mult, OP.add
            )
            nc.sync.dma_start(out_d[:], ith[0:BC, :])

    nc.compile()
    return nc


def _get_program(niter=NITER, a2a=True):
    key = ("nc", niter, a2a)
    if key not in _CACHE:
        _CACHE[key] = _build_program(niter, a2a)
    return _CACHE[key]


def make_in_maps(**inputs):
    stim = np.asarray(inputs["stimulus"], dtype=np.float32)
    dv = np.asarray(inputs["deltavf"], dtype=np.float32).reshape(B, 1)
    WJ1 = np.asarray(inputs["W_J_D1"], dtype=np.float32)
    WK1 = np.asarray(inputs["W_K_D1"], dtype=np.float32)
    WJ2 = np.asarray(inputs["W_J_D2"], dtype=np.float32)
    WK2 = np.asarray(inputs["W_K_D2"], dtype=np.float32)
    W1 = np.asarray(inputs["W_D1_GPi"], dtype=np.float32)
    W2 = np.asarray(inputs["W_D2_GPi"], dtype=np.float32)

    stimT = np.ascontiguousarray(stim.T)

    # group of partition p is the sample b_local = p % 16
    A = np.tile(np.eye(BC, dtype=np.float32), (CORES, CORES))
    I = np.eye(128, dtype=np.float32)
    consts = np.concatenate(
        [
            A,
            np.float32(EPS) * A,
            np.float32((1.0 + EPS) / 3.0) * A,
            np.float32(-(2.0 * EPS) / 3.0) * A,
            2.0 * I,
            np.float32(-EPS) * I,
            np.float32(2.0 / 3.0) * I,
            np.float32(2.0 * EPS / 3.0) * I,
            np.float32(-(4.0 + EPS) / 3.0) * I,
        ],
        axis=1,
    ).astype(np.float32)

    def gtile(wg):  # (2048,) -> [128, 256] in phase-2 layout (p = dh*16 + bl)
        return np.repeat(wg.reshape(CORES, DS), BC, axis=0).astype(np.float32)

    wgpi = np.concatenate(
        [gtile(W1[:, 0]), gtile(W1[:, 1]), gtile(W2[:, 0]), gtile(W2[:, 1])], axis=1
    )

    in_maps = []
    for c in range(CORES):
        sl = slice(c * DS, (c + 1) * DS)
        wcat = np.ascontiguousarray(
            np.concatenate([WJ1[:, sl], WK1[:, sl], WJ2[:, sl], WK2[:, sl]], axis=1)
        )
        dvf_c = np.tile(dv[c * BC : (c + 1) * BC, 0], CORES).reshape(128, 1)
        dvf_c = np.ascontiguousarray(dvf_c, dtype=np.float32)
        in_maps.append(
            {
                "stimT": stimT,
                "wcat": wcat,
                "consts": consts,
                "wgpi": wgpi,
                "dvf": dvf_c,
            }
        )
    return in_maps


def kernel(**inputs):
    from concourse import bass_utils

    # Degenerate exact-zero gates: J = K = 0 elementwise leaves every state
    # at 0 and the output at 0 (the closed-form FF would hit 0/0 there).
    stim = np.asarray(inputs["stimulus"])
    if not stim.any() or not (
        np.asarray(inputs["W_J_D1"]).any()
        or np.asarray(inputs["W_K_D1"]).any()
        or np.asarray(inputs["W_J_D2"]).any()
        or np.asarray(inputs["W_K_D2"]).any()
    ):
        return np.zeros((B, 2), np.float32)

    nc = _get_program()
    in_maps = make_in_maps(**inputs)
    res = bass_utils.run_bass_kernel_spmd(nc, in_maps, core_ids=list(range(CORES)))
    out = np.concatenate(
        [np.asarray(res.results[c]["out_ithal"]) for c in range(CORES)], axis=0
    )
    return out.astype(np.float32)
